# revision 35
# baseline (speedup 1.0000x reference)
import numpy as np

STACK, UNITS, D, EPS = 12, 4, 128, 1e-3
NPART = 128
T, NB = 16, 2048                 # t-blocks per group, cols per t-block
GROUPS = 2
G_ROWS = T * NB                  # 32768
NH = NB // 1024                  # rc rounds per stage (1024-col z tiles)


def _bf16(a):
    import ml_dtypes
    return np.asarray(a, dtype=ml_dtypes.bfloat16)


def prep_consts(inputs):
    """Host-side weight packing for the u-major T=16 layout."""
    ws = [np.asarray(inputs[f"w{i}"], np.float32) for i in range(STACK)]
    gamma = np.asarray(inputs["gamma"], np.float32)
    beta = np.asarray(inputs["beta"], np.float32)
    mean = np.asarray(inputs["mean"], np.float32)
    var = np.asarray(inputs["var"], np.float32)
    wf = np.asarray(inputs["wf"], np.float32)
    bf = np.asarray(inputs["bf"], np.float32)

    s = gamma / np.sqrt(var + EPS)
    bsh = beta - mean * s
    wd = wf[:, 0] - wf[:, 1]
    bd = float(bf[0] - bf[1])

    c = {}
    Wx = np.zeros((D, 49), np.float32)
    for i in range(STACK):
        for u in range(UNITS):
            Wx[:, 12 * u + i] = ws[i][4 * i:, u] * s[i, u]
    Wx[:, 48] = wd[48:]
    c["wx"] = _bf16(Wx)

    # A chunks: stage i, chunk cc = source stages {2cc, 2cc+1}
    for i in range(1, STACK):
        for cc in range(i // 2):
            M = np.zeros((128, 64), np.float32)
            for jj in range(2):
                j = 2 * cc + jj
                for v in range(4):
                    for u in range(4):
                        val = ws[i][4 * (i - 1 - j) + v, u] * s[i, u]
                        M[64 * jj + 16 * v:64 * jj + 16 * v + 16, 16 * u:16 * u + 16] \
                            [np.arange(16), np.arange(16)] = val
            c[f"a_{i}_{cc}"] = _bf16(M)
        if i % 2 == 1:
            j = i - 1
            M = np.zeros((64, 64), np.float32)
            for v in range(4):
                for u in range(4):
                    val = ws[i][4 * (i - 1 - j) + v, u] * s[i, u]
                    M[16 * v:16 * v + 16, 16 * u:16 * u + 16][np.arange(16), np.arange(16)] = val
            c[f"ap_{i}"] = _bf16(M)

    for cc in range(6):
        M = np.zeros((128, 64), np.float32)
        for jj in range(2):
            j = 2 * cc + jj
            for v in range(4):
                val = wd[4 * (11 - j) + v]
                for u in range(4):
                    M[64 * jj + 16 * v:64 * jj + 16 * v + 16, 16 * u:16 * u + 16] \
                        [np.arange(16), np.arange(16)] = val
        c[f"wd_{cc}"] = _bf16(M)

    c["s_id"] = _bf16(np.eye(64, dtype=np.float32))
    S_d = np.zeros((64, 64), np.float32)
    for t in range(T):
        for u in range(4):
            S_d[t, 16 * u + t] = 1.0
    c["s_d"] = _bf16(S_d)

    B = np.zeros((64, STACK), np.float32)
    for i in range(STACK):
        for u in range(4):
            B[16 * u:16 * u + 16, i] = bsh[i, u]
    c["bias"] = B
    c["bd"] = bd

    # pack the many small stationaries into 3 big tensors (3 DMA loads
    # instead of ~45 -- the per-DMA issue cost was delaying the first
    # scatters and with them the whole first recurrence)
    a_keys = [f"a_{i}_{cc}" for i in range(1, STACK) for cc in range(i // 2)]
    c2 = {"biga": np.concatenate([c.pop(k) for k in a_keys]
                                 + [c.pop(f"wd_{cc}") for cc in range(6)], axis=1)}
    s_keys = [f"ap_{i}" for i in range(1, STACK, 2)] + ["s_id", "s_d"]
    c2["bigs"] = np.concatenate([c.pop(k) for k in s_keys], axis=1)
    c2["wx"] = c.pop("wx")
    c2["bias"] = c.pop("bias")
    c2["bd"] = c.pop("bd")
    return c2


def build_kernel(ctx, tc, outs, ins, *, bd):
    import concourse.mybir as mybir

    nc = tc.nc
    f32 = mybir.dt.float32
    bf16 = mybir.dt.bfloat16
    ACT = mybir.ActivationFunctionType
    ALU = mybir.AluOpType

    x_ap = ins["x"]
    out_ap = outs["out"]

    const_pool = ctx.enter_context(tc.tile_pool(name="consts", bufs=1))

    def load_const(name, shape, dt=f32):
        t = const_pool.tile(list(shape), dt, tag=name, name=name)
        nc.gpsimd.dma_start(t[:], ins[name])
        return t

    wx_sb = load_const("wx", (D, 49), bf16)
    n_a = sum(i // 2 for i in range(1, STACK))            # 30
    biga = load_const("biga", (128, (n_a + 6) * 64), bf16)
    bigs = load_const("bigs", (64, 8 * 64), bf16)
    a_sb = {}
    k = 0
    for i in range(1, STACK):
        for cc in range(i // 2):
            a_sb[(i, cc)] = biga[:, k * 64:(k + 1) * 64]
            k += 1
    wd_sb = [biga[:, (n_a + cc) * 64:(n_a + cc + 1) * 64] for cc in range(6)]
    ap_sb = {i: bigs[:, k * 64:(k + 1) * 64]
             for k, i in enumerate(range(1, STACK, 2))}
    sid_sb = bigs[:, 6 * 64:7 * 64]
    sd_sb = bigs[:, 7 * 64:8 * 64]
    bias_sb = load_const("bias", (64, STACK))

    xt_pool = ctx.enter_context(tc.tile_pool(name="xt", bufs=3))
    cx_pool = ctx.enter_context(tc.tile_pool(name="cx", bufs=3))
    cxf_pool = ctx.enter_context(tc.tile_pool(name="cxf", bufs=2))
    cxfd_pool = ctx.enter_context(tc.tile_pool(name="cxfd", bufs=2))
    y2_pool = ctx.enter_context(tc.tile_pool(name="y2", bufs=2))
    out_pool = ctx.enter_context(tc.tile_pool(name="outsb", bufs=2))

    pcx_pool = ctx.enter_context(tc.tile_pool(name="pcx", bufs=2, space="PSUM"))
    z_pool = ctx.enter_context(tc.tile_pool(name="z", bufs=4, space="PSUM"))

    # Per-group state (tiles), created lazily by the pipeline below.
    state = {}

    def start_group(g):
        cxF = cxf_pool.tile([64, STACK * NB], bf16, tag="cxF", name="cxF")
        cxFd = cxfd_pool.tile([64, NB], bf16, tag="cxFd", name="cxFd")
        # rows 16..64 of cxFd feed the K=64 d-inject; zero everything first
        # (scatters then overwrite rows 0..15)
        nc.vector.memset(cxFd[:], 0.0)
        y2 = [y2_pool.tile([128, NB], bf16, tag=f"y2c{cc}", name=f"y2c{cc}")
              for cc in range(6)]
        state[g] = (cxF, cxFd, y2)

    def emit_xtile(g, t):
        cxF, cxFd, y2 = state[g]
        r0 = g * G_ROWS + t * NB
        xt = xt_pool.tile([NPART, NB], bf16, tag="xt")
        eng = nc.sync if t % 2 == 0 else nc.scalar
        eng.dma_start(xt[:], x_ap[r0:r0 + NB, :], transpose=True)
        cx = cx_pool.tile([49, NB], bf16, tag="cx")
        for rc in range(NH):
            pcx = pcx_pool.tile([49, 1024], f32, tag="pcx")
            for h in range(2):
                nc.tensor.matmul(
                    pcx[:, h * 512:(h + 1) * 512], wx_sb[:],
                    xt[:, rc * 1024 + h * 512: rc * 1024 + (h + 1) * 512],
                    start=True, stop=True,
                )
            dst = cx[:, rc * 1024:(rc + 1) * 1024]
            nc.vector.tensor_copy(dst, pcx[:])
        # one-DMA scatter: rows 12u+i -> cxF[16u+t, block i]
        ed = cxF[:].rearrange("(u s) (i n) -> u s i n", u=4, i=STACK)[:, t]
        nc.gpsimd.dma_start(ed, cx[0:48, :])
        nc.gpsimd.dma_start(cxFd[t:t + 1, :], cx[48:49, :])

    def emit_stage(g, i):
        cxF, cxFd, y2 = state[g]
        # per-rc z tiles (1 PSUM bank each): relu of the rc0 half can retire
        # while rc1 matmuls still run, unblocking stage i+1's rc0 chunks early
        zs = [z_pool.tile([128, 512], f32, tag="z", name=f"z{rc}")
              for rc in range(NH)]

        def slices(rc, cb):
            zsl = zs[rc][cb * 64:(cb + 1) * 64, :]
            ysl = slice(rc * 1024 + cb * 512, rc * 1024 + (cb + 1) * 512)
            return zsl, ysl

        ncc = i // 2
        has_part = (i % 2 == 1)
        # stationary-major emission: 4 matmuls (rc x cb) per weight load
        for rc in range(NH):
            for cb in range(2):
                zsl, ysl = slices(rc, cb)
                csl = slice(i * NB + rc * 1024 + cb * 512,
                            i * NB + rc * 1024 + (cb + 1) * 512)
                nc.tensor.matmul(zsl, sid_sb[:], cxF[0:64, csl],
                                 start=True, stop=(ncc == 0 and not has_part))
        if has_part:
            for rc in range(NH):
                for cb in range(2):
                    zsl, ysl = slices(rc, cb)
                    nc.tensor.matmul(zsl, ap_sb[i][:], y2[i // 2][0:64, ysl],
                                     start=False, stop=(ncc == 0))
        for cc in range(ncc):
            for rc in range(NH):
                for cb in range(2):
                    zsl, ysl = slices(rc, cb)
                    nc.tensor.matmul(zsl, a_sb[(i, cc)][:], y2[cc][:, ysl],
                                     start=False, stop=(cc == ncc - 1))
        # relu + bias -> y2 slice, per (rc, cb)
        ch, half = i // 2, 64 * (i % 2)
        for rc in range(NH):
            for cb in range(2):
                src = zs[rc][cb * 64:(cb + 1) * 64, :]
                dst = y2[ch][half:half + 64,
                             rc * 1024 + cb * 512: rc * 1024 + (cb + 1) * 512]
                if (i + rc + cb) % 2 == 0:
                    nc.scalar.activation(dst, src, ACT.Relu, bias=bias_sb[:, i:i + 1])
                else:
                    nc.vector.tensor_scalar(dst, src, bias_sb[:, i:i + 1], 0.0,
                                            ALU.add, ALU.max)

    def emit_tail(g):
        cxF, cxFd, y2 = state[g]
        pds = [z_pool.tile([128, 512], f32, tag="z", name=f"pd{rc}")
               for rc in range(NH)]
        for rc in range(NH):
            for cb in range(2):
                psl = pds[rc][cb * 64:(cb + 1) * 64, :]
                dsl = slice(rc * 1024 + cb * 512, rc * 1024 + (cb + 1) * 512)
                nc.tensor.matmul(psl, sd_sb[:], cxFd[0:64, dsl],
                                 start=True, stop=False)
        for cc in range(6):
            for rc in range(NH):
                for cb in range(2):
                    psl = pds[rc][cb * 64:(cb + 1) * 64, :]
                    ysl = slice(rc * 1024 + cb * 512, rc * 1024 + (cb + 1) * 512)
                    nc.tensor.matmul(psl, wd_sb[cc][:], y2[cc][:, ysl],
                                     start=False, stop=(cc == 5))
        outsb = out_pool.tile([128, NB], f32, tag="outsb")
        o4 = outsb[:].rearrange("p (rc n two) -> p rc n two", rc=NH, two=2)
        for rc in range(NH):
            nc.scalar.activation(o4[:, rc, :, 0], pds[rc][:], ACT.Sigmoid,
                                 bias=float(bd))
            nc.scalar.activation(o4[:, rc, :, 1], pds[rc][:], ACT.Sigmoid,
                                 bias=float(-bd), scale=-1.0)
        og = out_ap[g * G_ROWS:(g + 1) * G_ROWS, :].rearrange(
            "(t rc c n) two -> c t rc (n two)", rc=NH, c=2, n=512)
        for cb in range(2):
            osrc = outsb[cb * 64:cb * 64 + T, :].rearrange("p (rc f) -> p rc f", rc=NH)
            nc.gpsimd.dma_start(og[cb], osrc)

    # Software pipeline: group g's recurrence interleaves group g+1's x-tiles
    # so the PE never drains (keeps the HAM clock warm). Group g's tail (wd
    # chain) is deferred into group g+1's early stages for the same reason.
    start_group(0)
    for t in range(T):
        emit_xtile(0, t)
    pending_tail = None
    for g in range(GROUPS):
        if g + 1 < GROUPS:
            start_group(g + 1)
        emitted = 0
        for i in range(STACK):
            emit_stage(g, i)
            if pending_tail is not None:
                emit_tail(pending_tail)
                pending_tail = None
            if g + 1 < GROUPS:
                want = (i + 1) * T // STACK
                while emitted < want:
                    emit_xtile(g + 1, emitted)
                    emitted += 1
        pending_tail = g
    emit_tail(pending_tail)


# ---------------------------------------------------------------------------
# Self-contained entry point: kernel(**inputs) -> [500000, 2] float32
# ---------------------------------------------------------------------------

import sys as _sys
if '/opt/trn_rl_repo' not in _sys.path:
    _sys.path.insert(0, '/opt/trn_rl_repo')

B_FULL = 500000
N_CORES = 8
CORE_ROWS = GROUPS * G_ROWS                      # 65536
B_PAD = CORE_ROWS * N_CORES                      # 524288

_CACHE = {}


def _build_nc(const_shapes, bd):
    from contextlib import ExitStack
    import concourse.mybir as mybir
    from concourse import bacc
    import concourse.tile as tile

    nc = bacc.Bacc("TRN2", target_bir_lowering=False, debug=False,
                   num_devices=N_CORES)
    ins = {}
    ins["x"] = nc.dram_tensor("x", [CORE_ROWS, D], mybir.dt.bfloat16,
                              kind="ExternalInput").ap()
    for name, shp, npdt in const_shapes:
        dt = mybir.dt.bfloat16 if npdt == 'bfloat16' else mybir.dt.float32
        ins[name] = nc.dram_tensor(name, list(shp), dt,
                                   kind="ExternalInput").ap()
    outs = {"out": nc.dram_tensor("out", [CORE_ROWS, 2], mybir.dt.float32,
                                  kind="ExternalOutput").ap()}
    with tile.TileContext(nc) as tc:
        with ExitStack() as ctx:
            build_kernel(ctx, tc, outs, ins, bd=bd)
    nc.compile()
    return nc


def kernel(**inputs):
    import numpy as np
    import ml_dtypes
    from concourse.bass_utils import run_bass_kernel_spmd

    consts = prep_consts(inputs)
    bd = consts.pop("bd")
    x = np.asarray(inputs["x"], dtype=np.float32)
    assert x.shape == (B_FULL, D)
    xp = np.zeros((B_PAD, D), ml_dtypes.bfloat16)
    xp[:B_FULL] = x.astype(ml_dtypes.bfloat16)

    key = "nc"
    if key not in _CACHE:
        shapes = tuple((k, v.shape, str(v.dtype)) for k, v in consts.items())
        _CACHE[key] = _build_nc(shapes, bd)
    nc = _CACHE[key]

    in_maps = []
    for c in range(N_CORES):
        m = {"x": xp[c * CORE_ROWS:(c + 1) * CORE_ROWS]}
        m.update(consts)
        in_maps.append(m)
    res = run_bass_kernel_spmd(nc, in_maps, core_ids=list(range(N_CORES)))
    out = np.concatenate([res.results[c]["out"] for c in range(N_CORES)], axis=0)
    return out[:B_FULL]


# revision 36
# speedup vs baseline: 1.2193x; 1.2193x over previous
import numpy as np

STACK, UNITS, D, EPS = 12, 4, 128, 1e-3
NPART = 128
T, NB = 16, 2048                 # t-blocks per group, cols per t-block
GROUPS = 2
G_ROWS = T * NB                  # 32768
NH = NB // 1024                  # rc rounds per stage (1024-col z tiles)


def _bf16(a):
    import ml_dtypes
    return np.asarray(a, dtype=ml_dtypes.bfloat16)


def prep_consts(inputs):
    """Host-side weight packing for the u-major T=16 layout."""
    ws = [np.asarray(inputs[f"w{i}"], np.float32) for i in range(STACK)]
    gamma = np.asarray(inputs["gamma"], np.float32)
    beta = np.asarray(inputs["beta"], np.float32)
    mean = np.asarray(inputs["mean"], np.float32)
    var = np.asarray(inputs["var"], np.float32)
    wf = np.asarray(inputs["wf"], np.float32)
    bf = np.asarray(inputs["bf"], np.float32)

    s = gamma / np.sqrt(var + EPS)
    bsh = beta - mean * s
    wd = wf[:, 0] - wf[:, 1]
    bd = float(bf[0] - bf[1])

    c = {}
    Wx = np.zeros((D, 49), np.float32)
    for i in range(STACK):
        for u in range(UNITS):
            Wx[:, 12 * u + i] = ws[i][4 * i:, u] * s[i, u]
    Wx[:, 48] = wd[48:]
    c["wx"] = _bf16(Wx)

    # A chunks: stage i, chunk cc = source stages {2cc, 2cc+1}
    for i in range(1, STACK):
        for cc in range(i // 2):
            M = np.zeros((128, 64), np.float32)
            for jj in range(2):
                j = 2 * cc + jj
                for v in range(4):
                    for u in range(4):
                        val = ws[i][4 * (i - 1 - j) + v, u] * s[i, u]
                        M[64 * jj + 16 * v:64 * jj + 16 * v + 16, 16 * u:16 * u + 16] \
                            [np.arange(16), np.arange(16)] = val
            c[f"a_{i}_{cc}"] = _bf16(M)
        if i % 2 == 1:
            j = i - 1
            M = np.zeros((64, 64), np.float32)
            for v in range(4):
                for u in range(4):
                    val = ws[i][4 * (i - 1 - j) + v, u] * s[i, u]
                    M[16 * v:16 * v + 16, 16 * u:16 * u + 16][np.arange(16), np.arange(16)] = val
            c[f"ap_{i}"] = _bf16(M)

    for cc in range(6):
        M = np.zeros((128, 64), np.float32)
        for jj in range(2):
            j = 2 * cc + jj
            for v in range(4):
                val = wd[4 * (11 - j) + v]
                for u in range(4):
                    M[64 * jj + 16 * v:64 * jj + 16 * v + 16, 16 * u:16 * u + 16] \
                        [np.arange(16), np.arange(16)] = val
        c[f"wd_{cc}"] = _bf16(M)

    c["s_id"] = _bf16(np.eye(64, dtype=np.float32))
    S_d = np.zeros((64, 64), np.float32)
    for t in range(T):
        for u in range(4):
            S_d[t, 16 * u + t] = 1.0
    c["s_d"] = _bf16(S_d)

    B = np.zeros((64, STACK), np.float32)
    for i in range(STACK):
        for u in range(4):
            B[16 * u:16 * u + 16, i] = bsh[i, u]
    c["bias"] = B
    c["bd"] = bd

    # pack the many small stationaries into 3 big tensors (3 DMA loads
    # instead of ~45 -- the per-DMA issue cost was delaying the first
    # scatters and with them the whole first recurrence)
    a_keys = [f"a_{i}_{cc}" for i in range(1, STACK) for cc in range(i // 2)]
    c2 = {"biga": np.concatenate([c.pop(k) for k in a_keys]
                                 + [c.pop(f"wd_{cc}") for cc in range(6)], axis=1)}
    s_keys = [f"ap_{i}" for i in range(1, STACK, 2)] + ["s_id", "s_d"]
    c2["bigs"] = np.concatenate([c.pop(k) for k in s_keys], axis=1)
    c2["wx"] = c.pop("wx")
    c2["bias"] = c.pop("bias")
    c2["bd"] = c.pop("bd")
    return c2


def build_kernel(ctx, tc, outs, ins, *, bd):
    import concourse.mybir as mybir

    nc = tc.nc
    f32 = mybir.dt.float32
    bf16 = mybir.dt.bfloat16
    ACT = mybir.ActivationFunctionType
    ALU = mybir.AluOpType

    x_ap = ins["x"]
    out_ap = outs["out"]

    const_pool = ctx.enter_context(tc.tile_pool(name="consts", bufs=1))

    def load_const(name, shape, dt=f32):
        t = const_pool.tile(list(shape), dt, tag=name, name=name)
        nc.gpsimd.dma_start(t[:], ins[name])
        return t

    wx_sb = load_const("wx", (D, 49), bf16)
    n_a = sum(i // 2 for i in range(1, STACK))            # 30
    biga = load_const("biga", (128, (n_a + 6) * 64), bf16)
    bigs = load_const("bigs", (64, 8 * 64), bf16)
    a_sb = {}
    k = 0
    for i in range(1, STACK):
        for cc in range(i // 2):
            a_sb[(i, cc)] = biga[:, k * 64:(k + 1) * 64]
            k += 1
    wd_sb = [biga[:, (n_a + cc) * 64:(n_a + cc + 1) * 64] for cc in range(6)]
    ap_sb = {i: bigs[:, k * 64:(k + 1) * 64]
             for k, i in enumerate(range(1, STACK, 2))}
    sid_sb = bigs[:, 6 * 64:7 * 64]
    sd_sb = bigs[:, 7 * 64:8 * 64]
    bias_sb = load_const("bias", (64, STACK))

    xt_pool = ctx.enter_context(tc.tile_pool(name="xt", bufs=3))
    cx_pool = ctx.enter_context(tc.tile_pool(name="cx", bufs=3))
    cxf_pool = ctx.enter_context(tc.tile_pool(name="cxf", bufs=2))
    cxfd_pool = ctx.enter_context(tc.tile_pool(name="cxfd", bufs=2))
    y2_pool = ctx.enter_context(tc.tile_pool(name="y2", bufs=2))
    out_pool = ctx.enter_context(tc.tile_pool(name="outsb", bufs=2))

    pcx_pool = ctx.enter_context(tc.tile_pool(name="pcx", bufs=2, space="PSUM"))
    z_pool = ctx.enter_context(tc.tile_pool(name="z", bufs=4, space="PSUM"))

    # Per-group state (tiles), created lazily by the pipeline below.
    state = {}

    def start_group(g):
        cxF = cxf_pool.tile([64, STACK * NB], bf16, tag="cxF", name="cxF")
        cxFd = cxfd_pool.tile([64, NB], bf16, tag="cxFd", name="cxFd")
        # rows 16..64 of cxFd feed the K=64 d-inject; zero everything first
        # (scatters then overwrite rows 0..15)
        nc.vector.memset(cxFd[:], 0.0)
        y2 = [y2_pool.tile([128, NB], bf16, tag=f"y2c{cc}", name=f"y2c{cc}")
              for cc in range(6)]
        state[g] = (cxF, cxFd, y2)

    def emit_xtile(g, t):
        cxF, cxFd, y2 = state[g]
        r0 = g * G_ROWS + t * NB
        xt = xt_pool.tile([NPART, NB], bf16, tag="xt")
        eng = nc.sync if t % 2 == 0 else nc.scalar
        eng.dma_start(xt[:], x_ap[r0:r0 + NB, :], transpose=True)
        cx = cx_pool.tile([49, NB], bf16, tag="cx")
        for rc in range(NH):
            pcx = pcx_pool.tile([49, 1024], f32, tag="pcx")
            for h in range(2):
                nc.tensor.matmul(
                    pcx[:, h * 512:(h + 1) * 512], wx_sb[:],
                    xt[:, rc * 1024 + h * 512: rc * 1024 + (h + 1) * 512],
                    start=True, stop=True,
                )
            dst = cx[:, rc * 1024:(rc + 1) * 1024]
            if (t + rc) % 2 == 0:
                nc.scalar.activation(dst, pcx[:], ACT.Copy)
            else:
                nc.vector.tensor_copy(dst, pcx[:])
        # one-DMA scatter: rows 12u+i -> cxF[16u+t, block i]
        ed = cxF[:].rearrange("(u s) (i n) -> u s i n", u=4, i=STACK)[:, t]
        nc.gpsimd.dma_start(ed, cx[0:48, :])
        nc.scalar.dma_start(cxFd[t:t + 1, :], cx[48:49, :])

    def emit_stage(g, i):
        cxF, cxFd, y2 = state[g]
        # per-rc z tiles (1 PSUM bank each): relu of the rc0 half can retire
        # while rc1 matmuls still run, unblocking stage i+1's rc0 chunks early
        zs = [z_pool.tile([128, 512], f32, tag="z", name=f"z{rc}")
              for rc in range(NH)]

        def slices(rc, cb):
            zsl = zs[rc][cb * 64:(cb + 1) * 64, :]
            ysl = slice(rc * 1024 + cb * 512, rc * 1024 + (cb + 1) * 512)
            return zsl, ysl

        ncc = i // 2
        has_part = (i % 2 == 1)
        # stationary-major emission: 4 matmuls (rc x cb) per weight load
        for rc in range(NH):
            for cb in range(2):
                zsl, ysl = slices(rc, cb)
                csl = slice(i * NB + rc * 1024 + cb * 512,
                            i * NB + rc * 1024 + (cb + 1) * 512)
                nc.tensor.matmul(zsl, sid_sb[:], cxF[0:64, csl],
                                 start=True, stop=(ncc == 0 and not has_part))
        if has_part:
            for rc in range(NH):
                for cb in range(2):
                    zsl, ysl = slices(rc, cb)
                    nc.tensor.matmul(zsl, ap_sb[i][:], y2[i // 2][0:64, ysl],
                                     start=False, stop=(ncc == 0))
        for cc in range(ncc):
            for rc in range(NH):
                for cb in range(2):
                    zsl, ysl = slices(rc, cb)
                    nc.tensor.matmul(zsl, a_sb[(i, cc)][:], y2[cc][:, ysl],
                                     start=False, stop=(cc == ncc - 1))
        # relu + bias -> y2 slice, per (rc, cb)
        ch, half = i // 2, 64 * (i % 2)
        for rc in range(NH):
            for cb in range(2):
                src = zs[rc][cb * 64:(cb + 1) * 64, :]
                dst = y2[ch][half:half + 64,
                             rc * 1024 + cb * 512: rc * 1024 + (cb + 1) * 512]
                if (i + rc + cb) % 2 == 0:
                    nc.scalar.activation(dst, src, ACT.Relu, bias=bias_sb[:, i:i + 1])
                else:
                    nc.vector.tensor_scalar(dst, src, bias_sb[:, i:i + 1], 0.0,
                                            ALU.add, ALU.max)

    def emit_tail(g):
        cxF, cxFd, y2 = state[g]
        pds = [z_pool.tile([128, 512], f32, tag="z", name=f"pd{rc}")
               for rc in range(NH)]
        for rc in range(NH):
            for cb in range(2):
                psl = pds[rc][cb * 64:(cb + 1) * 64, :]
                dsl = slice(rc * 1024 + cb * 512, rc * 1024 + (cb + 1) * 512)
                nc.tensor.matmul(psl, sd_sb[:], cxFd[0:64, dsl],
                                 start=True, stop=False)
        for cc in range(6):
            for rc in range(NH):
                for cb in range(2):
                    psl = pds[rc][cb * 64:(cb + 1) * 64, :]
                    ysl = slice(rc * 1024 + cb * 512, rc * 1024 + (cb + 1) * 512)
                    nc.tensor.matmul(psl, wd_sb[cc][:], y2[cc][:, ysl],
                                     start=False, stop=(cc == 5))
        outsb = out_pool.tile([128, NB], f32, tag="outsb")
        o4 = outsb[:].rearrange("p (rc n two) -> p rc n two", rc=NH, two=2)
        for rc in range(NH):
            nc.scalar.activation(o4[:, rc, :, 0], pds[rc][:], ACT.Sigmoid,
                                 bias=float(bd))
            nc.scalar.activation(o4[:, rc, :, 1], pds[rc][:], ACT.Sigmoid,
                                 bias=float(-bd), scale=-1.0)
        og = out_ap[g * G_ROWS:(g + 1) * G_ROWS, :].rearrange(
            "(t rc c n) two -> c t rc (n two)", rc=NH, c=2, n=512)
        for cb in range(2):
            osrc = outsb[cb * 64:cb * 64 + T, :].rearrange("p (rc f) -> p rc f", rc=NH)
            nc.gpsimd.dma_start(og[cb], osrc)

    # Software pipeline: group g's recurrence interleaves group g+1's x-tiles
    # so the PE never drains (keeps the HAM clock warm). Group g's tail (wd
    # chain) is deferred into group g+1's early stages for the same reason.
    start_group(0)
    for t in range(T):
        emit_xtile(0, t)
    pending_tail = None
    for g in range(GROUPS):
        if g + 1 < GROUPS:
            start_group(g + 1)
        emitted = 0
        for i in range(STACK):
            emit_stage(g, i)
            if pending_tail is not None:
                emit_tail(pending_tail)
                pending_tail = None
            if g + 1 < GROUPS:
                want = (i + 1) * T // STACK
                while emitted < want:
                    emit_xtile(g + 1, emitted)
                    emitted += 1
        pending_tail = g
    emit_tail(pending_tail)


# ---------------------------------------------------------------------------
# Self-contained entry point: kernel(**inputs) -> [500000, 2] float32
# ---------------------------------------------------------------------------

import sys as _sys
if '/opt/trn_rl_repo' not in _sys.path:
    _sys.path.insert(0, '/opt/trn_rl_repo')

B_FULL = 500000
N_CORES = 8
CORE_ROWS = GROUPS * G_ROWS                      # 65536
B_PAD = CORE_ROWS * N_CORES                      # 524288

_CACHE = {}


def _build_nc(const_shapes, bd):
    from contextlib import ExitStack
    import concourse.mybir as mybir
    from concourse import bacc
    import concourse.tile as tile

    nc = bacc.Bacc("TRN2", target_bir_lowering=False, debug=False,
                   num_devices=N_CORES)
    ins = {}
    ins["x"] = nc.dram_tensor("x", [CORE_ROWS, D], mybir.dt.bfloat16,
                              kind="ExternalInput").ap()
    for name, shp, npdt in const_shapes:
        dt = mybir.dt.bfloat16 if npdt == 'bfloat16' else mybir.dt.float32
        ins[name] = nc.dram_tensor(name, list(shp), dt,
                                   kind="ExternalInput").ap()
    outs = {"out": nc.dram_tensor("out", [CORE_ROWS, 2], mybir.dt.float32,
                                  kind="ExternalOutput").ap()}
    with tile.TileContext(nc) as tc:
        with ExitStack() as ctx:
            build_kernel(ctx, tc, outs, ins, bd=bd)
    nc.compile()
    return nc


def kernel(**inputs):
    import numpy as np
    import ml_dtypes
    from concourse.bass_utils import run_bass_kernel_spmd

    consts = prep_consts(inputs)
    bd = consts.pop("bd")
    x = np.asarray(inputs["x"], dtype=np.float32)
    assert x.shape == (B_FULL, D)
    xp = np.zeros((B_PAD, D), ml_dtypes.bfloat16)
    xp[:B_FULL] = x.astype(ml_dtypes.bfloat16)

    key = "nc"
    if key not in _CACHE:
        shapes = tuple((k, v.shape, str(v.dtype)) for k, v in consts.items())
        _CACHE[key] = _build_nc(shapes, bd)
    nc = _CACHE[key]

    in_maps = []
    for c in range(N_CORES):
        m = {"x": xp[c * CORE_ROWS:(c + 1) * CORE_ROWS]}
        m.update(consts)
        in_maps.append(m)
    res = run_bass_kernel_spmd(nc, in_maps, core_ids=list(range(N_CORES)))
    out = np.concatenate([res.results[c]["out"] for c in range(N_CORES)], axis=0)
    return out[:B_FULL]


# revision 38
# speedup vs baseline: 1.2237x; 1.0035x over previous
import numpy as np

STACK, UNITS, D, EPS = 12, 4, 128, 1e-3
NPART = 128
T, NB = 16, 2048                 # t-blocks per group, cols per t-block
GROUPS = 2
G_ROWS = T * NB                  # 32768
NH = NB // 1024                  # rc rounds per stage (1024-col z tiles)


def _bf16(a):
    import ml_dtypes
    return np.asarray(a, dtype=ml_dtypes.bfloat16)


def prep_consts(inputs):
    """Host-side weight packing for the u-major T=16 layout."""
    ws = [np.asarray(inputs[f"w{i}"], np.float32) for i in range(STACK)]
    gamma = np.asarray(inputs["gamma"], np.float32)
    beta = np.asarray(inputs["beta"], np.float32)
    mean = np.asarray(inputs["mean"], np.float32)
    var = np.asarray(inputs["var"], np.float32)
    wf = np.asarray(inputs["wf"], np.float32)
    bf = np.asarray(inputs["bf"], np.float32)

    s = gamma / np.sqrt(var + EPS)
    bsh = beta - mean * s
    wd = wf[:, 0] - wf[:, 1]
    bd = float(bf[0] - bf[1])

    c = {}
    Wx = np.zeros((D, 49), np.float32)
    for i in range(STACK):
        for u in range(UNITS):
            Wx[:, 12 * u + i] = ws[i][4 * i:, u] * s[i, u]
    Wx[:, 48] = wd[48:]
    c["wx"] = _bf16(Wx)

    # A chunks: stage i, chunk cc = source stages {2cc, 2cc+1}
    for i in range(1, STACK):
        for cc in range(i // 2):
            M = np.zeros((128, 64), np.float32)
            for jj in range(2):
                j = 2 * cc + jj
                for v in range(4):
                    for u in range(4):
                        val = ws[i][4 * (i - 1 - j) + v, u] * s[i, u]
                        M[64 * jj + 16 * v:64 * jj + 16 * v + 16, 16 * u:16 * u + 16] \
                            [np.arange(16), np.arange(16)] = val
            c[f"a_{i}_{cc}"] = _bf16(M)
        if i % 2 == 1:
            j = i - 1
            M = np.zeros((64, 64), np.float32)
            for v in range(4):
                for u in range(4):
                    val = ws[i][4 * (i - 1 - j) + v, u] * s[i, u]
                    M[16 * v:16 * v + 16, 16 * u:16 * u + 16][np.arange(16), np.arange(16)] = val
            c[f"ap_{i}"] = _bf16(M)

    for cc in range(6):
        M = np.zeros((128, 64), np.float32)
        for jj in range(2):
            j = 2 * cc + jj
            for v in range(4):
                val = wd[4 * (11 - j) + v]
                for u in range(4):
                    M[64 * jj + 16 * v:64 * jj + 16 * v + 16, 16 * u:16 * u + 16] \
                        [np.arange(16), np.arange(16)] = val
        c[f"wd_{cc}"] = _bf16(M)

    c["s_id"] = _bf16(np.eye(64, dtype=np.float32))
    S_d = np.zeros((64, 64), np.float32)
    for t in range(T):
        for u in range(4):
            S_d[t, 16 * u + t] = 1.0
    c["s_d"] = _bf16(S_d)

    B = np.zeros((64, STACK), np.float32)
    for i in range(STACK):
        for u in range(4):
            B[16 * u:16 * u + 16, i] = bsh[i, u]
    c["bias"] = B
    c["bd"] = bd

    # pack the many small stationaries into 3 big tensors (3 DMA loads
    # instead of ~45 -- the per-DMA issue cost was delaying the first
    # scatters and with them the whole first recurrence)
    a_keys = [f"a_{i}_{cc}" for i in range(1, STACK) for cc in range(i // 2)]
    c2 = {"biga": np.concatenate([c.pop(k) for k in a_keys]
                                 + [c.pop(f"wd_{cc}") for cc in range(6)], axis=1)}
    s_keys = [f"ap_{i}" for i in range(1, STACK, 2)] + ["s_id", "s_d"]
    c2["bigs"] = np.concatenate([c.pop(k) for k in s_keys], axis=1)
    c2["wx"] = c.pop("wx")
    c2["bias"] = c.pop("bias")
    c2["bd"] = c.pop("bd")
    return c2


def build_kernel(ctx, tc, outs, ins, *, bd):
    import concourse.mybir as mybir

    nc = tc.nc
    f32 = mybir.dt.float32
    bf16 = mybir.dt.bfloat16
    ACT = mybir.ActivationFunctionType
    ALU = mybir.AluOpType

    x_ap = ins["x"]
    out_ap = outs["out"]

    const_pool = ctx.enter_context(tc.tile_pool(name="consts", bufs=1))

    def load_const(name, shape, dt=f32):
        t = const_pool.tile(list(shape), dt, tag=name, name=name)
        nc.gpsimd.dma_start(t[:], ins[name])
        return t

    wx_sb = load_const("wx", (D, 49), bf16)
    n_a = sum(i // 2 for i in range(1, STACK))            # 30
    biga = load_const("biga", (128, (n_a + 6) * 64), bf16)
    bigs = load_const("bigs", (64, 8 * 64), bf16)
    a_sb = {}
    k = 0
    for i in range(1, STACK):
        for cc in range(i // 2):
            a_sb[(i, cc)] = biga[:, k * 64:(k + 1) * 64]
            k += 1
    wd_sb = [biga[:, (n_a + cc) * 64:(n_a + cc + 1) * 64] for cc in range(6)]
    ap_sb = {i: bigs[:, k * 64:(k + 1) * 64]
             for k, i in enumerate(range(1, STACK, 2))}
    sid_sb = bigs[:, 6 * 64:7 * 64]
    sd_sb = bigs[:, 7 * 64:8 * 64]
    bias_sb = load_const("bias", (64, STACK))

    xt_pool = ctx.enter_context(tc.tile_pool(name="xt", bufs=3))
    cx_pool = ctx.enter_context(tc.tile_pool(name="cx", bufs=3))
    cxf_pool = ctx.enter_context(tc.tile_pool(name="cxf", bufs=2))
    cxfd_pool = ctx.enter_context(tc.tile_pool(name="cxfd", bufs=2))
    y2_pool = ctx.enter_context(tc.tile_pool(name="y2", bufs=2))
    out_pool = ctx.enter_context(tc.tile_pool(name="outsb", bufs=2))

    pcx_pool = ctx.enter_context(tc.tile_pool(name="pcx", bufs=2, space="PSUM"))
    z_pool = ctx.enter_context(tc.tile_pool(name="z", bufs=4, space="PSUM"))

    # Per-group state (tiles), created lazily by the pipeline below.
    state = {}

    def start_group(g):
        cxF = cxf_pool.tile([64, STACK * NB], bf16, tag="cxF", name="cxF")
        cxFd = cxfd_pool.tile([64, NB], bf16, tag="cxFd", name="cxFd")
        # rows 16..64 of cxFd feed the K=64 d-inject; zero everything first
        # (scatters then overwrite rows 0..15)
        nc.vector.memset(cxFd[:], 0.0)
        y2 = [y2_pool.tile([128, NB], bf16, tag=f"y2c{cc}", name=f"y2c{cc}")
              for cc in range(6)]
        state[g] = (cxF, cxFd, y2)

    def emit_xtile(g, t):
        cxF, cxFd, y2 = state[g]
        r0 = g * G_ROWS + t * NB
        xt = xt_pool.tile([NPART, NB], bf16, tag="xt")
        eng = nc.sync if t % 2 == 0 else nc.scalar
        eng.dma_start(xt[:], x_ap[r0:r0 + NB, :], transpose=True)
        cx = cx_pool.tile([49, NB], bf16, tag="cx")
        for rc in range(NH):
            pcx = pcx_pool.tile([49, 1024], f32, tag="pcx")
            for h in range(2):
                nc.tensor.matmul(
                    pcx[:, h * 512:(h + 1) * 512], wx_sb[:],
                    xt[:, rc * 1024 + h * 512: rc * 1024 + (h + 1) * 512],
                    start=True, stop=True,
                )
            dst = cx[:, rc * 1024:(rc + 1) * 1024]
            if (t + rc) % 2 == 0:
                nc.scalar.activation(dst, pcx[:], ACT.Copy)
            else:
                nc.vector.tensor_copy(dst, pcx[:])
        # one-DMA scatter: rows 12u+i -> cxF[16u+t, block i]
        ed = cxF[:].rearrange("(u s) (i n) -> u s i n", u=4, i=STACK)[:, t]
        nc.gpsimd.dma_start(ed, cx[0:48, :])
        nc.scalar.dma_start(cxFd[t:t + 1, :], cx[48:49, :])

    def emit_stage(g, i):
        cxF, cxFd, y2 = state[g]
        # per-rc z tiles (1 PSUM bank each): relu of the rc0 half can retire
        # while rc1 matmuls still run, unblocking stage i+1's rc0 chunks early
        zs = [z_pool.tile([128, 512], f32, tag="z", name=f"z{rc}")
              for rc in range(NH)]

        def slices(rc, cb):
            zsl = zs[rc][cb * 64:(cb + 1) * 64, :]
            ysl = slice(rc * 1024 + cb * 512, rc * 1024 + (cb + 1) * 512)
            return zsl, ysl

        ncc = i // 2
        has_part = (i % 2 == 1)
        # stationary-major emission: 4 matmuls (rc x cb) per weight load
        for rc in range(NH):
            for cb in range(2):
                zsl, ysl = slices(rc, cb)
                csl = slice(i * NB + rc * 1024 + cb * 512,
                            i * NB + rc * 1024 + (cb + 1) * 512)
                nc.tensor.matmul(zsl, sid_sb[:], cxF[0:64, csl],
                                 start=True, stop=(ncc == 0 and not has_part))
        if has_part:
            for rc in range(NH):
                for cb in range(2):
                    zsl, ysl = slices(rc, cb)
                    nc.tensor.matmul(zsl, ap_sb[i][:], y2[i // 2][0:64, ysl],
                                     start=False, stop=(ncc == 0))
        for cc in range(ncc):
            for rc in range(NH):
                for cb in range(2):
                    zsl, ysl = slices(rc, cb)
                    nc.tensor.matmul(zsl, a_sb[(i, cc)][:], y2[cc][:, ysl],
                                     start=False, stop=(cc == ncc - 1))
        # relu + bias -> y2 slice, per (rc, cb)
        ch, half = i // 2, 64 * (i % 2)
        for rc in range(NH):
            for cb in range(2):
                src = zs[rc][cb * 64:(cb + 1) * 64, :]
                dst = y2[ch][half:half + 64,
                             rc * 1024 + cb * 512: rc * 1024 + (cb + 1) * 512]
                if (i + rc + cb) % 2 == 0:
                    nc.scalar.activation(dst, src, ACT.Relu, bias=bias_sb[:, i:i + 1])
                else:
                    nc.vector.tensor_scalar(dst, src, bias_sb[:, i:i + 1], 0.0,
                                            ALU.add, ALU.max)

    def emit_tail(g):
        cxF, cxFd, y2 = state[g]
        pds = [z_pool.tile([128, 512], f32, tag="z", name=f"pd{rc}")
               for rc in range(NH)]
        for rc in range(NH):
            for cb in range(2):
                psl = pds[rc][cb * 64:(cb + 1) * 64, :]
                dsl = slice(rc * 1024 + cb * 512, rc * 1024 + (cb + 1) * 512)
                nc.tensor.matmul(psl, sd_sb[:], cxFd[0:64, dsl],
                                 start=True, stop=False)
        for cc in range(6):
            for rc in range(NH):
                for cb in range(2):
                    psl = pds[rc][cb * 64:(cb + 1) * 64, :]
                    ysl = slice(rc * 1024 + cb * 512, rc * 1024 + (cb + 1) * 512)
                    nc.tensor.matmul(psl, wd_sb[cc][:], y2[cc][:, ysl],
                                     start=False, stop=(cc == 5))
        outsb = out_pool.tile([128, NB], f32, tag="outsb")
        o4 = outsb[:].rearrange("p (rc n two) -> p rc n two", rc=NH, two=2)
        for rc in range(NH):
            nc.scalar.activation(o4[:, rc, :, 0], pds[rc][:], ACT.Sigmoid,
                                 bias=float(bd))
            nc.scalar.activation(o4[:, rc, :, 1], pds[rc][:], ACT.Sigmoid,
                                 bias=float(-bd), scale=-1.0)
        og = out_ap[g * G_ROWS:(g + 1) * G_ROWS, :].rearrange(
            "(t rc c n) two -> c t rc (n two)", rc=NH, c=2, n=512)
        for cb in range(2):
            osrc = outsb[cb * 64:cb * 64 + T, :].rearrange("p (rc f) -> p rc f", rc=NH)
            nc.gpsimd.dma_start(og[cb], osrc)

    # Software pipeline: group g's recurrence interleaves group g+1's x-tiles
    # so the PE never drains (keeps the HAM clock warm). Group g's tail (wd
    # chain) is deferred into group g+1's early stages for the same reason.
    start_group(0)
    for t in range(T):
        emit_xtile(0, t)
    pending_tail = None
    for g in range(GROUPS):
        if g + 1 < GROUPS:
            start_group(g + 1)
        emitted = 0
        for i in range(STACK):
            emit_stage(g, i)
            if pending_tail is not None:
                emit_tail(pending_tail)
                pending_tail = None
            if g + 1 < GROUPS:
                want = (i + 1) * T // STACK
                while emitted < want:
                    emit_xtile(g + 1, emitted)
                    emitted += 1
        pending_tail = g
    emit_tail(pending_tail)


# ---------------------------------------------------------------------------
# Self-contained entry point: kernel(**inputs) -> [500000, 2] float32
# ---------------------------------------------------------------------------

import sys as _sys
if '/opt/trn_rl_repo' not in _sys.path:
    _sys.path.insert(0, '/opt/trn_rl_repo')

B_FULL = 500000
N_CORES = 8
CORE_ROWS = GROUPS * G_ROWS                      # 65536
B_PAD = CORE_ROWS * N_CORES                      # 524288

_CACHE = {}


def _build_nc(const_shapes, bd):
    from contextlib import ExitStack
    import concourse.mybir as mybir
    from concourse import bacc
    import concourse.tile as tile

    nc = bacc.Bacc("TRN2", target_bir_lowering=False, debug=False,
                   num_devices=N_CORES)
    ins = {}
    ins["x"] = nc.dram_tensor("x", [CORE_ROWS, D], mybir.dt.bfloat16,
                              kind="ExternalInput").ap()
    for name, shp, npdt in const_shapes:
        dt = mybir.dt.bfloat16 if npdt == 'bfloat16' else mybir.dt.float32
        ins[name] = nc.dram_tensor(name, list(shp), dt,
                                   kind="ExternalInput").ap()
    outs = {"out": nc.dram_tensor("out", [CORE_ROWS, 2], mybir.dt.float32,
                                  kind="ExternalOutput").ap()}
    with tile.TileContext(nc) as tc:
        with ExitStack() as ctx:
            build_kernel(ctx, tc, outs, ins, bd=bd)
    nc.compile()
    return nc


def kernel(**inputs):
    import numpy as np
    import ml_dtypes
    from concourse.bass_utils import run_bass_kernel_spmd

    consts = prep_consts(inputs)
    bd = consts.pop("bd")
    x = np.asarray(inputs["x"], dtype=np.float32)
    assert x.shape == (B_FULL, D)
    xp = np.zeros((B_PAD, D), ml_dtypes.bfloat16)
    xp[:B_FULL] = x.astype(ml_dtypes.bfloat16)

    key = "nc"
    if key not in _CACHE:
        shapes = tuple((k, v.shape, str(v.dtype)) for k, v in consts.items())
        _CACHE[key] = _build_nc(shapes, bd)
    nc = _CACHE[key]

    in_maps = []
    for c in range(N_CORES):
        m = {"x": xp[c * CORE_ROWS:(c + 1) * CORE_ROWS]}
        m.update(consts)
        in_maps.append(m)
    res = run_bass_kernel_spmd(nc, in_maps, core_ids=list(range(N_CORES)))
    out = np.concatenate([res.results[c]["out"] for c in range(N_CORES)], axis=0)
    return out[:B_FULL]


# revision 39
# speedup vs baseline: 1.2665x; 1.0350x over previous
import numpy as np

STACK, UNITS, D, EPS = 12, 4, 128, 1e-3
NPART = 128
T, NB = 16, 2048                 # t-blocks per group, cols per t-block
GROUPS = 2
G_ROWS = T * NB                  # 32768
NH = NB // 1024                  # rc rounds per stage (1024-col z tiles)


def _bf16(a):
    import ml_dtypes
    return np.asarray(a, dtype=ml_dtypes.bfloat16)


def prep_consts(inputs):
    """Host-side weight packing for the u-major T=16 layout."""
    ws = [np.asarray(inputs[f"w{i}"], np.float32) for i in range(STACK)]
    gamma = np.asarray(inputs["gamma"], np.float32)
    beta = np.asarray(inputs["beta"], np.float32)
    mean = np.asarray(inputs["mean"], np.float32)
    var = np.asarray(inputs["var"], np.float32)
    wf = np.asarray(inputs["wf"], np.float32)
    bf = np.asarray(inputs["bf"], np.float32)

    s = gamma / np.sqrt(var + EPS)
    bsh = beta - mean * s
    wd = wf[:, 0] - wf[:, 1]
    bd = float(bf[0] - bf[1])

    c = {}
    Wx = np.zeros((D, 49), np.float32)
    for i in range(STACK):
        for u in range(UNITS):
            Wx[:, 12 * u + i] = ws[i][4 * i:, u] * s[i, u]
    Wx[:, 48] = wd[48:]
    c["wx"] = _bf16(Wx)

    # A chunks: stage i, chunk cc = source stages {2cc, 2cc+1}
    for i in range(1, STACK):
        for cc in range(i // 2):
            M = np.zeros((128, 64), np.float32)
            for jj in range(2):
                j = 2 * cc + jj
                for v in range(4):
                    for u in range(4):
                        val = ws[i][4 * (i - 1 - j) + v, u] * s[i, u]
                        M[64 * jj + 16 * v:64 * jj + 16 * v + 16, 16 * u:16 * u + 16] \
                            [np.arange(16), np.arange(16)] = val
            c[f"a_{i}_{cc}"] = _bf16(M)
        if i % 2 == 1:
            j = i - 1
            M = np.zeros((64, 64), np.float32)
            for v in range(4):
                for u in range(4):
                    val = ws[i][4 * (i - 1 - j) + v, u] * s[i, u]
                    M[16 * v:16 * v + 16, 16 * u:16 * u + 16][np.arange(16), np.arange(16)] = val
            c[f"ap_{i}"] = _bf16(M)

    for cc in range(6):
        M = np.zeros((128, 64), np.float32)
        for jj in range(2):
            j = 2 * cc + jj
            for v in range(4):
                val = wd[4 * (11 - j) + v]
                for u in range(4):
                    M[64 * jj + 16 * v:64 * jj + 16 * v + 16, 16 * u:16 * u + 16] \
                        [np.arange(16), np.arange(16)] = val
        c[f"wd_{cc}"] = _bf16(M)

    c["s_id"] = _bf16(np.eye(64, dtype=np.float32))
    S_d = np.zeros((64, 64), np.float32)
    for t in range(T):
        for u in range(4):
            S_d[t, 16 * u + t] = 1.0
    c["s_d"] = _bf16(S_d)

    B = np.zeros((64, STACK), np.float32)
    for i in range(STACK):
        for u in range(4):
            B[16 * u:16 * u + 16, i] = bsh[i, u]
    c["bias"] = B
    c["bd"] = bd

    # pack the many small stationaries into 3 big tensors (3 DMA loads
    # instead of ~45 -- the per-DMA issue cost was delaying the first
    # scatters and with them the whole first recurrence)
    a_keys = [f"a_{i}_{cc}" for i in range(1, STACK) for cc in range(i // 2)]
    c2 = {"biga": np.concatenate([c.pop(k) for k in a_keys]
                                 + [c.pop(f"wd_{cc}") for cc in range(6)], axis=1)}
    s_keys = [f"ap_{i}" for i in range(1, STACK, 2)] + ["s_id", "s_d"]
    c2["bigs"] = np.concatenate([c.pop(k) for k in s_keys], axis=1)
    c2["wx"] = c.pop("wx")
    c2["bias"] = c.pop("bias")
    c2["bd"] = c.pop("bd")
    return c2


def build_kernel(ctx, tc, outs, ins, *, bd):
    import concourse.mybir as mybir

    nc = tc.nc
    f32 = mybir.dt.float32
    bf16 = mybir.dt.bfloat16
    ACT = mybir.ActivationFunctionType
    ALU = mybir.AluOpType

    x_ap = ins["x"]
    out_ap = outs["out"]

    const_pool = ctx.enter_context(tc.tile_pool(name="consts", bufs=1))

    def load_const(name, shape, dt=f32):
        t = const_pool.tile(list(shape), dt, tag=name, name=name)
        nc.gpsimd.dma_start(t[:], ins[name])
        return t

    wx_sb = load_const("wx", (D, 49), bf16)
    n_a = sum(i // 2 for i in range(1, STACK))            # 30
    biga = load_const("biga", (128, (n_a + 6) * 64), bf16)
    bigs = load_const("bigs", (64, 8 * 64), bf16)
    a_sb = {}
    k = 0
    for i in range(1, STACK):
        for cc in range(i // 2):
            a_sb[(i, cc)] = biga[:, k * 64:(k + 1) * 64]
            k += 1
    wd_sb = [biga[:, (n_a + cc) * 64:(n_a + cc + 1) * 64] for cc in range(6)]
    ap_sb = {i: bigs[:, k * 64:(k + 1) * 64]
             for k, i in enumerate(range(1, STACK, 2))}
    sid_sb = bigs[:, 6 * 64:7 * 64]
    sd_sb = bigs[:, 7 * 64:8 * 64]
    bias_sb = load_const("bias", (64, STACK))

    xt_pool = ctx.enter_context(tc.tile_pool(name="xt", bufs=4))
    cx_pool = ctx.enter_context(tc.tile_pool(name="cx", bufs=4))
    cxf_pool = ctx.enter_context(tc.tile_pool(name="cxf", bufs=2))
    cxfd_pool = ctx.enter_context(tc.tile_pool(name="cxfd", bufs=2))
    y2_pool = ctx.enter_context(tc.tile_pool(name="y2", bufs=2))
    out_pool = ctx.enter_context(tc.tile_pool(name="outsb", bufs=2))

    pcx_pool = ctx.enter_context(tc.tile_pool(name="pcx", bufs=2, space="PSUM"))
    z_pool = ctx.enter_context(tc.tile_pool(name="z", bufs=4, space="PSUM"))

    # Per-group state (tiles), created lazily by the pipeline below.
    state = {}

    def start_group(g):
        cxF = cxf_pool.tile([64, STACK * NB], bf16, tag="cxF", name="cxF")
        cxFd = cxfd_pool.tile([64, NB], bf16, tag="cxFd", name="cxFd")
        # rows 16..64 of cxFd feed the K=64 d-inject; zero everything first
        # (scatters then overwrite rows 0..15)
        nc.vector.memset(cxFd[:], 0.0)
        y2 = [y2_pool.tile([128, NB], bf16, tag=f"y2c{cc}", name=f"y2c{cc}")
              for cc in range(6)]
        state[g] = (cxF, cxFd, y2)

    def emit_xtile(g, t):
        cxF, cxFd, y2 = state[g]
        r0 = g * G_ROWS + t * NB
        xt = xt_pool.tile([NPART, NB], bf16, tag="xt")
        eng = nc.sync if t % 2 == 0 else nc.scalar
        eng.dma_start(xt[:], x_ap[r0:r0 + NB, :], transpose=True)
        cx = cx_pool.tile([49, NB], bf16, tag="cx")
        for rc in range(NH):
            pcx = pcx_pool.tile([49, 1024], f32, tag="pcx")
            for h in range(2):
                nc.tensor.matmul(
                    pcx[:, h * 512:(h + 1) * 512], wx_sb[:],
                    xt[:, rc * 1024 + h * 512: rc * 1024 + (h + 1) * 512],
                    start=True, stop=True,
                )
            dst = cx[:, rc * 1024:(rc + 1) * 1024]
            if (t + rc) % 2 == 0:
                nc.scalar.activation(dst, pcx[:], ACT.Copy)
            else:
                nc.vector.tensor_copy(dst, pcx[:])
        # one-DMA scatter: rows 12u+i -> cxF[16u+t, block i]
        ed = cxF[:].rearrange("(u s) (i n) -> u s i n", u=4, i=STACK)[:, t]
        nc.gpsimd.dma_start(ed, cx[0:48, :])
        nc.scalar.dma_start(cxFd[t:t + 1, :], cx[48:49, :])

    def emit_stage(g, i):
        cxF, cxFd, y2 = state[g]
        # per-rc z tiles (1 PSUM bank each): relu of the rc0 half can retire
        # while rc1 matmuls still run, unblocking stage i+1's rc0 chunks early
        zs = [z_pool.tile([128, 512], f32, tag="z", name=f"z{rc}")
              for rc in range(NH)]

        def slices(rc, cb):
            zsl = zs[rc][cb * 64:(cb + 1) * 64, :]
            ysl = slice(rc * 1024 + cb * 512, rc * 1024 + (cb + 1) * 512)
            return zsl, ysl

        ncc = i // 2
        has_part = (i % 2 == 1)
        # stationary-major emission: 4 matmuls (rc x cb) per weight load
        for rc in range(NH):
            for cb in range(2):
                zsl, ysl = slices(rc, cb)
                csl = slice(i * NB + rc * 1024 + cb * 512,
                            i * NB + rc * 1024 + (cb + 1) * 512)
                nc.tensor.matmul(zsl, sid_sb[:], cxF[0:64, csl],
                                 start=True, stop=(ncc == 0 and not has_part))
        if has_part:
            for rc in range(NH):
                for cb in range(2):
                    zsl, ysl = slices(rc, cb)
                    nc.tensor.matmul(zsl, ap_sb[i][:], y2[i // 2][0:64, ysl],
                                     start=False, stop=(ncc == 0))
        for cc in range(ncc):
            for rc in range(NH):
                for cb in range(2):
                    zsl, ysl = slices(rc, cb)
                    nc.tensor.matmul(zsl, a_sb[(i, cc)][:], y2[cc][:, ysl],
                                     start=False, stop=(cc == ncc - 1))
        # relu + bias -> y2 slice, per (rc, cb)
        ch, half = i // 2, 64 * (i % 2)
        for rc in range(NH):
            for cb in range(2):
                src = zs[rc][cb * 64:(cb + 1) * 64, :]
                dst = y2[ch][half:half + 64,
                             rc * 1024 + cb * 512: rc * 1024 + (cb + 1) * 512]
                if (i + rc + cb) % 2 == 0:
                    nc.scalar.activation(dst, src, ACT.Relu, bias=bias_sb[:, i:i + 1])
                else:
                    nc.vector.tensor_scalar(dst, src, bias_sb[:, i:i + 1], 0.0,
                                            ALU.add, ALU.max)

    def emit_tail(g):
        cxF, cxFd, y2 = state[g]
        pds = [z_pool.tile([128, 512], f32, tag="z", name=f"pd{rc}")
               for rc in range(NH)]
        for rc in range(NH):
            for cb in range(2):
                psl = pds[rc][cb * 64:(cb + 1) * 64, :]
                dsl = slice(rc * 1024 + cb * 512, rc * 1024 + (cb + 1) * 512)
                nc.tensor.matmul(psl, sd_sb[:], cxFd[0:64, dsl],
                                 start=True, stop=False)
        for cc in range(6):
            for rc in range(NH):
                for cb in range(2):
                    psl = pds[rc][cb * 64:(cb + 1) * 64, :]
                    ysl = slice(rc * 1024 + cb * 512, rc * 1024 + (cb + 1) * 512)
                    nc.tensor.matmul(psl, wd_sb[cc][:], y2[cc][:, ysl],
                                     start=False, stop=(cc == 5))
        outsb = out_pool.tile([128, NB], f32, tag="outsb")
        o4 = outsb[:].rearrange("p (rc n two) -> p rc n two", rc=NH, two=2)
        for rc in range(NH):
            nc.scalar.activation(o4[:, rc, :, 0], pds[rc][:], ACT.Sigmoid,
                                 bias=float(bd))
            nc.scalar.activation(o4[:, rc, :, 1], pds[rc][:], ACT.Sigmoid,
                                 bias=float(-bd), scale=-1.0)
        og = out_ap[g * G_ROWS:(g + 1) * G_ROWS, :].rearrange(
            "(t rc c n) two -> c t rc (n two)", rc=NH, c=2, n=512)
        for cb in range(2):
            osrc = outsb[cb * 64:cb * 64 + T, :].rearrange("p (rc f) -> p rc f", rc=NH)
            nc.gpsimd.dma_start(og[cb], osrc)

    # Software pipeline: group g's recurrence interleaves group g+1's x-tiles
    # so the PE never drains (keeps the HAM clock warm). Group g's tail (wd
    # chain) is deferred into group g+1's early stages for the same reason.
    start_group(0)
    for t in range(T):
        emit_xtile(0, t)
    pending_tail = None
    for g in range(GROUPS):
        if g + 1 < GROUPS:
            start_group(g + 1)
        emitted = 0
        for i in range(STACK):
            emit_stage(g, i)
            if pending_tail is not None:
                emit_tail(pending_tail)
                pending_tail = None
            if g + 1 < GROUPS:
                want = (i + 1) * T // STACK
                while emitted < want:
                    emit_xtile(g + 1, emitted)
                    emitted += 1
        pending_tail = g
    emit_tail(pending_tail)


# ---------------------------------------------------------------------------
# Self-contained entry point: kernel(**inputs) -> [500000, 2] float32
# ---------------------------------------------------------------------------

import sys as _sys
if '/opt/trn_rl_repo' not in _sys.path:
    _sys.path.insert(0, '/opt/trn_rl_repo')

B_FULL = 500000
N_CORES = 8
CORE_ROWS = GROUPS * G_ROWS                      # 65536
B_PAD = CORE_ROWS * N_CORES                      # 524288

_CACHE = {}


def _build_nc(const_shapes, bd):
    from contextlib import ExitStack
    import concourse.mybir as mybir
    from concourse import bacc
    import concourse.tile as tile

    nc = bacc.Bacc("TRN2", target_bir_lowering=False, debug=False,
                   num_devices=N_CORES)
    ins = {}
    ins["x"] = nc.dram_tensor("x", [CORE_ROWS, D], mybir.dt.bfloat16,
                              kind="ExternalInput").ap()
    for name, shp, npdt in const_shapes:
        dt = mybir.dt.bfloat16 if npdt == 'bfloat16' else mybir.dt.float32
        ins[name] = nc.dram_tensor(name, list(shp), dt,
                                   kind="ExternalInput").ap()
    outs = {"out": nc.dram_tensor("out", [CORE_ROWS, 2], mybir.dt.float32,
                                  kind="ExternalOutput").ap()}
    with tile.TileContext(nc) as tc:
        with ExitStack() as ctx:
            build_kernel(ctx, tc, outs, ins, bd=bd)
    nc.compile()
    return nc


def kernel(**inputs):
    import numpy as np
    import ml_dtypes
    from concourse.bass_utils import run_bass_kernel_spmd

    consts = prep_consts(inputs)
    bd = consts.pop("bd")
    x = np.asarray(inputs["x"], dtype=np.float32)
    assert x.shape == (B_FULL, D)
    xp = np.zeros((B_PAD, D), ml_dtypes.bfloat16)
    xp[:B_FULL] = x.astype(ml_dtypes.bfloat16)

    key = "nc"
    if key not in _CACHE:
        shapes = tuple((k, v.shape, str(v.dtype)) for k, v in consts.items())
        _CACHE[key] = _build_nc(shapes, bd)
    nc = _CACHE[key]

    in_maps = []
    for c in range(N_CORES):
        m = {"x": xp[c * CORE_ROWS:(c + 1) * CORE_ROWS]}
        m.update(consts)
        in_maps.append(m)
    res = run_bass_kernel_spmd(nc, in_maps, core_ids=list(range(N_CORES)))
    out = np.concatenate([res.results[c]["out"] for c in range(N_CORES)], axis=0)
    return out[:B_FULL]
